# revision 1
# baseline (speedup 1.0000x reference)
import math
from contextlib import ExitStack

import numpy as np

N, T, D, H = 512, 128, 512, 512
NC = 8
n = N // NC          # 64 samples per core
H4 = 4 * H           # 2048
SCALE = 1.0 / math.sqrt(H)

_cache = {}


def _build_kernel():
    if "nc" in _cache:
        return _cache["nc"]

    import concourse.bass as bass
    import concourse.tile as tile
    from concourse import bacc, mybir

    f32 = mybir.dt.float32
    bf16 = mybir.dt.bfloat16
    ALU = mybir.AluOpType
    ACTF = mybir.ActivationFunctionType
    AX = mybir.AxisListType

    nc = bacc.Bacc(
        "TRN2",
        target_bir_lowering=False,
        debug=False,
        enable_asserts=False,
        num_devices=NC,
    )

    # host-preprocessed layouts (see kernel() below)
    xh = nc.dram_tensor("xh", (T, 128, 256), bf16, kind="ExternalInput").ap()
    A2d = nc.dram_tensor("A2d", (128, 8 * H), bf16, kind="ExternalInput").ap()
    A2bd = nc.dram_tensor("A2bd", (128, H * 8), bf16, kind="ExternalInput").ap()
    Wc = nc.dram_tensor("Wc", (128, 12 * H4), bf16, kind="ExternalInput").ap()
    bvec = nc.dram_tensor("bvec", (1, H4), bf16, kind="ExternalInput").ap()
    identd = nc.dram_tensor("identd", (n, n), bf16, kind="ExternalInput").ap()
    onesd = nc.dram_tensor("onesd", (1, n), bf16, kind="ExternalInput").ap()
    hs = nc.dram_tensor("hs", (T, n, H), bf16, kind="ExternalOutput").ap()

    with tile.TileContext(nc) as tc, ExitStack() as ctx:
        const_pool = ctx.enter_context(tc.tile_pool(name="const", bufs=1))
        xts_pool = ctx.enter_context(tc.tile_pool(name="xts", bufs=3))
        psum_mm = ctx.enter_context(tc.tile_pool(name="psum_mm", bufs=2, space="PSUM"))
        psum_tp = ctx.enter_context(tc.tile_pool(name="psum_tp", bufs=4, space="PSUM"))

        # ---- persistent tiles ------------------------------------------------
        W_sb = const_pool.tile([128, 12 * H4], bf16)
        b_sb = const_pool.tile([1, H4], bf16)
        id_sb = const_pool.tile([n, n], bf16)
        ones_row = const_pool.tile([1, n], bf16)
        A2 = const_pool.tile([128, 8 * H], bf16)     # q=(pb,s); free=(pl, h)
        A2b = const_pool.tile([128, H * 8], bf16)    # q=(pb,s); free=(h, pl)
        nc.sync.dma_start(W_sb[:], Wc[:])
        nc.sync.dma_start(b_sb[:], bvec[:])
        nc.sync.dma_start(id_sb[:], identd[:])
        nc.sync.dma_start(ones_row[:], onesd[:])
        nc.sync.dma_start(A2[:], A2d[:])
        nc.sync.dma_start(A2b[:], A2bd[:])

        h2 = const_pool.tile([128, H], bf16)         # h duplicated on both halves
        hT = const_pool.tile([128, 4 * n], bf16)     # h^T: chunk ci at cols 64ci
        attnT = const_pool.tile([128, 4 * n], bf16)
        c_st = const_pool.tile([n, H], bf16)

        # per-step scratch (persistent; deps handled by tile framework)
        dotscr = const_pool.tile([128, H], bf16)
        dot_sb = const_pool.tile([128, 8], f32)
        th = const_pool.tile([128, 8], f32)
        thp1 = const_pool.tile([128, 8], bf16)
        om = const_pool.tile([128, 8], f32)
        rin = const_pool.tile([128, 8], f32)
        wexp = const_pool.tile([128, 8], bf16)
        s8 = const_pool.tile([128, 1], f32)
        s8hi_c = const_pool.tile([n, 1], f32)
        ssum = const_pool.tile([n, 1], f32)
        rs = const_pool.tile([n, 1], f32)
        prodA = const_pool.tile([128, H * 8], bf16)  # (h, pl) layout
        tr4 = const_pool.tile([128, H * 4], bf16)
        tr2 = const_pool.tile([128, H * 2], bf16)
        attn2 = const_pool.tile([128, H], bf16)
        attn2hi_c = const_pool.tile([n, H], bf16)
        attn_ps = const_pool.tile([n, H], bf16)
        attn_sm = const_pool.tile([n, H], bf16)
        gi = const_pool.tile([n, H], bf16)
        gf = const_pool.tile([n, H], bf16)
        go = const_pool.tile([n, H], bf16)
        gg = const_pool.tile([n, H], bf16)
        t1 = const_pool.tile([n, H], bf16)
        t2 = const_pool.tile([n, H], bf16)
        tct = const_pool.tile([n, H], bf16)
        h0p = const_pool.tile([128, H], f32)
        h0phi_c = const_pool.tile([n, H], f32)
        h0h = const_pool.tile([n, H], f32)

        # ---- h0 = mean over p of A_flat; c0 = h0 -----------------------------
        nc.vector.tensor_reduce(
            h0p[:], A2b[:].rearrange("q (h pl) -> q h pl", pl=8),
            axis=AX.X, op=ALU.add)
        nc.vector.tensor_copy(h0phi_c[:], h0p[n:128, :])
        nc.vector.tensor_tensor(h0h[:], h0p[0:n, :], h0phi_c[:], ALU.add)
        nc.scalar.activation(c_st[:], h0h[:], ACTF.Copy, scale=1.0 / 16.0)
        nc.scalar.activation(h2[0:n, :], h0h[:], ACTF.Copy, scale=1.0 / 16.0)
        nc.vector.tensor_copy(h2[n:128, :], h2[0:n, :])
        for ci in range(4):
            pt = psum_tp.tile([128, n], bf16)
            nc.tensor.transpose(pt[:], h2[0:n, 128 * ci:128 * (ci + 1)], id_sb[:])
            nc.scalar.copy(hT[:, n * ci:n * (ci + 1)], pt[:])

        # prefetch x for t=0,1
        xts_tiles = {}
        for tpre in range(2):
            xt0 = xts_pool.tile([128, 4 * n], bf16)
            nc.sync.dma_start(xt0[:], xh[tpre])
            xts_tiles[tpre] = xt0
        ps_tiles = {}

        # col-tiled matmul layout: psum tile ps[128, 1024]
        #   ps[0:64,   0:512]  = hbar cols    0:512  (i)   j=0, tile (0,0)
        #   ps[64:128, 0:512]  = hbar cols  512:1024 (f)   j=1, tile (0,64)
        #   ps[0:64, 512:1024] = hbar cols 1024:1536 (o)   j=2, tile (0,0)
        #   ps[64:128,512:1024]= hbar cols 1536:2048 (g)   j=3, tile (0,64)
        def mm(ps, c, j, lhsT, start, stop):
            rhs = (b_sb[:, 512 * j:512 * (j + 1)] if c == 12 else
                   W_sb[:, H4 * c + 512 * j:H4 * c + 512 * (j + 1)])
            lo = (j % 2 == 0)
            out = (ps[0:n, 512 * (j // 2):512 * (j // 2 + 1)] if lo else
                   ps[n:128, 512 * (j // 2):512 * (j // 2 + 1)])
            nc.tensor.matmul(out, lhsT, rhs, start=start, stop=stop,
                             tile_position=(0, 0) if lo else (0, n),
                             skip_group_check=True)

        # x-part of step 0 opens the accumulation groups for ps_tiles[0]
        ps0 = psum_mm.tile([128, 2 * 512], f32, tag="psmm")
        xts0 = xts_tiles.pop(0)
        for c in [8, 9, 10, 11]:
            lhsT0 = xts0[:, n * (c - 8):n * (c - 7)]
            for j in range(4):
                mm(ps0, c, j, lhsT0, start=(c == 8), stop=False)
        ps_tiles[0] = ps0

        # ---- recurrence ------------------------------------------------------
        for t in range(T):
            if t + 2 < T:
                xtp = xts_pool.tile([128, 4 * n], bf16)
                nc.sync.dma_start(xtp[:], xh[t + 2])
                xts_tiles[t + 2] = xtp
            ps = ps_tiles.pop(t)

            # -- phase A matmuls (h, bias); x was done during step t-1 ---------
            for c in [0, 1, 2, 3, 12]:
                lhsT = hT[:, n * c:n * (c + 1)] if c < 4 else ones_row[:]
                for j in range(4):
                    mm(ps, c, j, lhsT, start=False, stop=False)

            # -- attention: dot[s,p] = sum_h A[s,p,h]*h[s,h] (fused stt) -------
            for pl in range(8):
                nc.vector.scalar_tensor_tensor(
                    out=dotscr[:], in0=A2[:, H * pl:H * (pl + 1)], scalar=SCALE,
                    in1=h2[:], op0=ALU.mult, op1=ALU.mult,
                    accum_out=dot_sb[:, pl:pl + 1])
            # softmax via exp(x) = (1+tanh(x/2))/(1-tanh(x/2)); norm folded below
            nc.scalar.activation(th[:], dot_sb[:], ACTF.Tanh, scale=0.5)
            nc.scalar.activation(thp1[:], th[:], ACTF.Copy, bias=1.0)
            nc.scalar.activation(om[:], th[:], ACTF.Copy, bias=1.0, scale=-1.0)
            nc.vector.reciprocal(rin[:], om[:])
            nc.vector.tensor_tensor(wexp[:], thp1[:], rin[:], ALU.mult)
            nc.vector.tensor_reduce(s8[:], wexp[:], axis=AX.X, op=ALU.add)
            nc.vector.tensor_copy(s8hi_c[:], s8[n:128, :])
            nc.vector.tensor_tensor(ssum[:], s8[0:n, :], s8hi_c[:], ALU.add)
            nc.vector.reciprocal(rs[:], ssum[:])

            # -- x-part matmuls for step t+1 keep the PE busy during attention -
            if t + 1 < T:
                ps_next = psum_mm.tile([128, 2 * 512], f32, tag="psmm")
                xtsn = xts_tiles.pop(t + 1)
                for c in [8, 9, 10, 11]:
                    lhsTn = xtsn[:, n * (c - 8):n * (c - 7)]
                    for j in range(4):
                        mm(ps_next, c, j, lhsTn, start=(c == 8), stop=False)
                ps_tiles[t + 1] = ps_next

            # attn[s,h] = (sum_p wexp[s,p]*A[s,p,h]) / ssum[s]
            # pipelined per 128-wide h-chunk: tree-add -> scale -> transpose -> MM
            w_b = wexp[:].rearrange("q (r pl) -> q r pl", r=1).broadcast_to(
                [128, 128, 8])
            for ci in range(4):
                hsl = slice(1024 * ci, 1024 * (ci + 1))
                pA = prodA[:, hsl].rearrange("q (h pl) -> q h pl", pl=8)
                nc.vector.tensor_tensor(
                    pA, A2b[:, hsl].rearrange("q (h pl) -> q h pl", pl=8),
                    w_b, ALU.mult)
                v4 = tr4[:, 512 * ci:512 * (ci + 1)].rearrange(
                    "q (h pl) -> q h pl", pl=4)
                v2 = tr2[:, 256 * ci:256 * (ci + 1)].rearrange(
                    "q (h pl) -> q h pl", pl=2)
                nc.vector.tensor_tensor(v4, pA[:, :, 0:4], pA[:, :, 4:8],
                                        ALU.add)
                nc.vector.tensor_tensor(v2, v4[:, :, 0:2], v4[:, :, 2:4],
                                        ALU.add)
                csl = slice(128 * ci, 128 * (ci + 1))
                nc.vector.tensor_tensor(
                    attn2[:, csl].rearrange("q (h r) -> q h r", r=1),
                    v2[:, :, 0:1], v2[:, :, 1:2], ALU.add)
                nc.vector.tensor_copy(attn2hi_c[:, csl], attn2[n:128, csl])
                nc.vector.tensor_tensor(attn_ps[:, csl], attn2[0:n, csl],
                                        attn2hi_c[:, csl], ALU.add)
                nc.scalar.activation(attn_sm[:, csl], attn_ps[:, csl],
                                     ACTF.Copy, scale=rs[:])
                pt = psum_tp.tile([128, n], bf16)
                nc.tensor.transpose(pt[:], attn_sm[:, csl], id_sb[:])
                nc.scalar.copy(attnT[:, n * ci:n * (ci + 1)], pt[:])
                c = 4 + ci
                lhsT = attnT[:, n * ci:n * (ci + 1)]
                mm(ps, c, 0, lhsT, start=False, stop=(c == 7))
                mm(ps, c, 1, lhsT, start=False, stop=(c == 7))

            # -- gates (pipelined against remaining phase B matmuls) -----------
            nc.scalar.activation(gi[:], ps[0:n, 0:512], ACTF.Sigmoid)
            nc.scalar.activation(gf[:], ps[n:128, 0:512], ACTF.Sigmoid)
            nc.vector.tensor_tensor(t1[:], gf[:], c_st[:], ALU.mult)
            for c in [4, 5, 6, 7]:
                lhsT = attnT[:, n * (c - 4):n * (c - 3)]
                mm(ps, c, 2, lhsT, start=False, stop=(c == 7))
                mm(ps, c, 3, lhsT, start=False, stop=(c == 7))
            nc.scalar.activation(gg[:], ps[n:128, 512:1024], ACTF.Tanh)
            nc.scalar.activation(go[:], ps[0:n, 512:1024], ACTF.Sigmoid)
            nc.vector.tensor_tensor(t2[:], gi[:], gg[:], ALU.mult)
            nc.vector.tensor_tensor(c_st[:], t1[:], t2[:], ALU.add)
            nc.scalar.activation(tct[:], c_st[:], ACTF.Tanh)
            nc.vector.tensor_tensor(h2[0:n, :], go[:], tct[:], ALU.mult)
            nc.vector.tensor_copy(h2[n:128, :], h2[0:n, :])

            nc.gpsimd.dma_start(hs[t], h2[0:n, :])

            for ci in range(4):
                pt = psum_tp.tile([128, n], bf16)
                nc.tensor.transpose(pt[:], h2[0:n, 128 * ci:128 * (ci + 1)],
                                    id_sb[:])
                nc.scalar.copy(hT[:, n * ci:n * (ci + 1)], pt[:])

    nc.compile()
    _cache["nc"] = nc
    return nc


LAST_RESULT = None


def kernel(x, A, Wx, Wh, Wattn, b):
    import os
    import ml_dtypes
    from concourse import bass_utils

    nc = _build_kernel()
    bft = ml_dtypes.bfloat16

    Wcat = np.concatenate([np.asarray(Wh), np.asarray(Wattn), np.asarray(Wx)],
                          axis=0)                         # (1536, 2048)
    Wc_host = np.ascontiguousarray(
        Wcat.reshape(12, 128, H4).transpose(1, 0, 2).reshape(128, 12 * H4)
    ).astype(bft)
    b_host = np.asarray(b, dtype=np.float32).reshape(1, H4).astype(bft)
    ident = np.eye(n, dtype=np.float32).astype(bft)
    ones_h = np.ones((1, n), dtype=bft)

    in_maps = []
    for k in range(NC):
        xc = np.asarray(x[n * k:n * (k + 1)], dtype=np.float32)   # (64, T, D)
        Ac = np.asarray(A[n * k:n * (k + 1)], dtype=np.float32)   # (64, H, 4, 4)
        xh_host = np.ascontiguousarray(
            xc.transpose(1, 2, 0).reshape(T, 4, 128, n)
            .transpose(0, 2, 1, 3).reshape(T, 128, 4 * n)).astype(bft)
        A_flat = Ac.reshape(n, H, 16).transpose(0, 2, 1)          # (n, 16, H)
        A4 = A_flat.reshape(n, 2, 8, H)
        A2_host = np.ascontiguousarray(
            A4.transpose(1, 0, 2, 3).reshape(128, 8 * H)).astype(bft)
        A2b_host = np.ascontiguousarray(
            A4.transpose(1, 0, 3, 2).reshape(128, H * 8)).astype(bft)
        in_maps.append({
            "xh": xh_host,
            "A2d": A2_host,
            "A2bd": A2b_host,
            "Wc": Wc_host,
            "bvec": b_host,
            "identd": ident,
            "onesd": ones_h,
        })

    trace = os.environ.get("KERNEL_TRACE") == "1"
    tmpdir = os.environ.get("KERNEL_TRACE_DIR") or None
    res = bass_utils.run_bass_kernel_spmd(
        nc, in_maps, core_ids=list(range(NC)), trace=trace, tmpdir=tmpdir
    )
    global LAST_RESULT
    LAST_RESULT = res

    out = np.empty((N, T, H), dtype=np.float32)
    for k in range(NC):
        hs_k = np.asarray(res.results[k]["hs"])           # (T, n, H) bf16
        out[n * k:n * (k + 1)] = hs_k.transpose(1, 0, 2).astype(np.float32)
    return out



# revision 10
# speedup vs baseline: 1.1204x; 1.1204x over previous
import math
from contextlib import ExitStack

import numpy as np

N, T, D, H = 512, 128, 512, 512
NC = 8
n = N // NC          # 64 samples per core
H4 = 4 * H           # 2048
SCALE = 1.0 / math.sqrt(H)

# QUANT: "bf16" or "fp8" (fp8 = DoubleRow for Wh/Wattn/Wx matmuls)
QUANT = "bf16"
S_A = 8.0 if QUANT == "fp8" else 1.0    # stationary (activations) scale
S_W = 16.0 if QUANT == "fp8" else 1.0   # weight scale
QS = 1.0 / (S_A * S_W)                  # gate descale

_cache = {}


def _build_kernel():
    if "nc" in _cache:
        return _cache["nc"]

    import concourse.bass as bass
    import concourse.tile as tile
    from concourse import bacc, mybir

    f32 = mybir.dt.float32
    bf16 = mybir.dt.bfloat16
    fp8 = mybir.dt.float8e4
    qdt = fp8 if QUANT == "fp8" else bf16
    ALU = mybir.AluOpType
    ACTF = mybir.ActivationFunctionType
    AX = mybir.AxisListType
    DR = mybir.MatmulPerfMode.DoubleRow if QUANT == "fp8" else None

    nc = bacc.Bacc(
        "TRN2",
        target_bir_lowering=False,
        debug=False,
        enable_asserts=False,
        num_devices=NC,
    )

    xhd = nc.dram_tensor("xh", (T, 128, 256), qdt, kind="ExternalInput").ap()
    A2d = nc.dram_tensor("A2d", (128, 8 * H), bf16, kind="ExternalInput").ap()
    Wcd = nc.dram_tensor("Wc", (128, 12 * H4), qdt, kind="ExternalInput").ap()
    bd = nc.dram_tensor("bvec", (1, H4), bf16, kind="ExternalInput").ap()
    idd = nc.dram_tensor("identd", (n, n), bf16, kind="ExternalInput").ap()
    id2d = nc.dram_tensor("id2d", (128, n), bf16, kind="ExternalInput").ap()
    onesd = nc.dram_tensor("onesd", (1, n), bf16, kind="ExternalInput").ap()
    hs = nc.dram_tensor("hs", (T, n, H), bf16, kind="ExternalOutput").ap()

    with tile.TileContext(nc) as tc, ExitStack() as ctx:
        const_pool = ctx.enter_context(tc.tile_pool(name="const", bufs=1))
        xq_pool = ctx.enter_context(tc.tile_pool(name="xq", bufs=3))
        gps_pool = ctx.enter_context(tc.tile_pool(name="gps", bufs=2, space="PSUM"))
        tp_pool = ctx.enter_context(tc.tile_pool(name="tp", bufs=3, space="PSUM"))
        aps_pool = ctx.enter_context(tc.tile_pool(name="aps", bufs=1, space="PSUM"))

        # ---- persistent tiles --------------------------------------------
        W_sb = const_pool.tile([128, 12 * H4], qdt)
        b_sb = const_pool.tile([1, H4], bf16)
        id_sb = const_pool.tile([n, n], bf16)
        id2 = const_pool.tile([128, n], bf16)
        ones_row = const_pool.tile([1, n], bf16)
        A2 = const_pool.tile([128, 8 * H], bf16)     # q=(ph,s); free=(pl, h)
        nc.sync.dma_start(W_sb[:], Wcd[:])
        nc.sync.dma_start(b_sb[:], bd[:])
        nc.sync.dma_start(id_sb[:], idd[:])
        nc.sync.dma_start(id2[:], id2d[:])
        nc.sync.dma_start(ones_row[:], onesd[:])
        nc.sync.dma_start(A2[:], A2d[:])

        h2 = const_pool.tile([128, H], bf16)         # h duplicated on both halves
        c_st = const_pool.tile([n, H], bf16)
        hT = const_pool.tile([128, 4 * n], qdt)      # h^T chunks (x S_A)
        attnT = const_pool.tile([128, 4 * n], qdt)   # attn^T chunks (x S_A)

        # per-step scratch
        P = const_pool.tile([128, 8 * H], bf16)      # A2 * h products
        Q1 = const_pool.tile([128, 2048], bf16)      # h-fold tree levels
        Q2 = const_pool.tile([128, 1024], bf16)
        Q3 = const_pool.tile([128, 512], bf16)
        Q4 = const_pool.tile([128, 256], bf16)
        Q5 = const_pool.tile([128, 128], bf16)
        Q6 = const_pool.tile([128, 64], bf16)
        Q7 = const_pool.tile([128, 32], bf16)
        Q8 = const_pool.tile([128, 16], bf16)
        dot8 = const_pool.tile([128, 8], f32)
        th = const_pool.tile([128, 8], f32)
        thp1 = const_pool.tile([128, 8], bf16)
        om = const_pool.tile([128, 8], f32)
        rin = const_pool.tile([128, 8], f32)
        wexp = const_pool.tile([128, 8], f32)
        Dg = const_pool.tile([128, 8 * n], bf16)     # 8 diag(w_p-pair) tiles
        s8 = const_pool.tile([128, 1], f32)
        s8c = const_pool.tile([n, 1], f32)
        ssum = const_pool.tile([n, 1], f32)
        rs = const_pool.tile([n, 1], f32)
        attn_s = const_pool.tile([n, H], bf16)
        gi = const_pool.tile([n, H], bf16)
        gf = const_pool.tile([n, H], bf16)
        go = const_pool.tile([n, H], bf16)
        gg = const_pool.tile([n, H], bf16)
        t1 = const_pool.tile([n, H], bf16)
        t2 = const_pool.tile([n, H], bf16)
        tct = const_pool.tile([n, H], bf16)
        i0 = const_pool.tile([128, 4 * H], f32)      # h0 init scratch
        i1 = const_pool.tile([128, 2 * H], f32)
        h0h = const_pool.tile([n, H], f32)

        # gate-packed psum quadrants: j=0 i -> [0:64, 0:512]; j=1 f ->
        # [64:128, 0:512]; j=2 o -> [0:64, 512:1024]; j=3 g -> [64:128,512:1024]
        def quad(ps, j):
            lo = j in (0, 2)
            rows = slice(0, n) if lo else slice(n, 128)
            cols = slice(512 * (j // 2), 512 * (j // 2) + 512)
            return ps[rows, cols], ((0, 0) if lo else (0, n))

        def mm_w(ps, m, j, lhsT8, start, stop):
            # weight matmul against matrix m (0=Wh, 1=Wattn, 2=Wx)
            out, tpos = quad(ps, j)
            if QUANT == "fp8":
                for cp in range(2):          # chunk-pairs, DoubleRow
                    rhs = W_sb[:, H4 * 4 * m + H4 * 2 * cp:
                               H4 * 4 * m + H4 * 2 * (cp + 1)].rearrange(
                        "q (t c) -> q t c", t=2)[:, :, 512 * j:512 * (j + 1)]
                    lhsT = lhsT8[:, 128 * cp:128 * (cp + 1)].rearrange(
                        "q (t w) -> q t w", t=2)
                    nc.tensor.matmul(out, lhsT, rhs,
                                     start=(start and cp == 0),
                                     stop=(stop and cp == 1),
                                     perf_mode=DR, tile_position=tpos,
                                     skip_group_check=True)
            else:
                for c in range(4):
                    rhs = W_sb[:, H4 * (4 * m + c) + 512 * j:
                               H4 * (4 * m + c) + 512 * (j + 1)]
                    lhsT = lhsT8[:, n * c:n * (c + 1)]
                    nc.tensor.matmul(out, lhsT, rhs,
                                     start=(start and c == 0),
                                     stop=(stop and c == 3),
                                     tile_position=tpos,
                                     skip_group_check=True)

        def mm_bias(ps, j):
            out, tpos = quad(ps, j)
            nc.tensor.matmul(out, ones_row[:], b_sb[:, 512 * j:512 * (j + 1)],
                             start=True, stop=False, tile_position=tpos,
                             skip_group_check=True)

        # ---- h0 = mean over p of A_flat; c0 = h0 -------------------------
        nc.vector.tensor_tensor(i0[:], A2[:, 0:2048], A2[:, 2048:4096], ALU.add)
        nc.vector.tensor_tensor(i1[:], i0[:, 0:1024], i0[:, 1024:2048], ALU.add)
        nc.vector.tensor_tensor(i0[:, 0:512], i1[:, 0:512], i1[:, 512:1024],
                                ALU.add)
        nc.vector.tensor_copy(i1[0:n, 0:512], i0[n:128, 0:512])
        nc.vector.tensor_tensor(h0h[:], i0[0:n, 0:512], i1[0:n, 0:512],
                                ALU.add)
        nc.scalar.activation(c_st[:], h0h[:], ACTF.Copy, scale=1.0 / 16.0)
        nc.scalar.activation(h2[0:n, :], h0h[:], ACTF.Copy, scale=1.0 / 16.0)
        nc.vector.tensor_copy(h2[n:128, :], h2[0:n, :])
        for ci in range(4):
            pt = tp_pool.tile([128, n], bf16)
            nc.tensor.transpose(pt[:], h2[0:n, 128 * ci:128 * (ci + 1)],
                                id_sb[:])
            nc.scalar.activation(hT[:, n * ci:n * (ci + 1)], pt[:], ACTF.Copy,
                                 scale=S_A)

        # prefetch x for t=0,1
        xts = {}
        for tpre in range(2):
            xt0 = xq_pool.tile([128, 256], qdt)
            nc.sync.dma_start(xt0[:], xhd[tpre])
            xts[tpre] = xt0

        JORD = (3, 2, 0, 1)   # g, o, i, f
        Pv = P[:].rearrange("q (pl h) -> q pl h", pl=8)

        # ---- recurrence ---------------------------------------------------
        for t in range(T):
            ps = gps_pool.tile([128, 2 * 512], f32, tag="gps")
            xt = xts.pop(t)

            # phase A: bias + Wh + x matmuls (gate-ordered)
            for j in JORD:
                mm_bias(ps, j)
                mm_w(ps, 0, j, hT, start=False, stop=False)
                mm_w(ps, 2, j, xt, start=False, stop=False)

            # attention dot: P = A2 * h2 (broadcast over pl), fold h by tree
            nc.vector.tensor_tensor(
                Pv, A2[:].rearrange("q (pl h) -> q pl h", pl=8),
                h2[:].rearrange("q (r h) -> q r h", r=1).broadcast_to(
                    [128, 8, H]),
                ALU.mult)
            nc.vector.tensor_tensor(
                Q1[:].rearrange("q (pl h) -> q pl h", pl=8),
                Pv[:, :, 0:256], Pv[:, :, 256:512], ALU.add)
            nc.vector.tensor_tensor(
                Q2[:].rearrange("q (pl h) -> q pl h", pl=8),
                Q1[:].rearrange("q (pl h) -> q pl h", pl=8)[:, :, 0:128],
                Q1[:].rearrange("q (pl h) -> q pl h", pl=8)[:, :, 128:256],
                ALU.add)
            nc.vector.tensor_tensor(
                Q3[:].rearrange("q (pl h) -> q pl h", pl=8),
                Q2[:].rearrange("q (pl h) -> q pl h", pl=8)[:, :, 0:64],
                Q2[:].rearrange("q (pl h) -> q pl h", pl=8)[:, :, 64:128],
                ALU.add)
            nc.vector.tensor_tensor(
                Q4[:].rearrange("q (pl h) -> q pl h", pl=8),
                Q3[:].rearrange("q (pl h) -> q pl h", pl=8)[:, :, 0:32],
                Q3[:].rearrange("q (pl h) -> q pl h", pl=8)[:, :, 32:64],
                ALU.add)
            nc.vector.tensor_tensor(
                Q5[:].rearrange("q (pl h) -> q pl h", pl=8),
                Q4[:].rearrange("q (pl h) -> q pl h", pl=8)[:, :, 0:16],
                Q4[:].rearrange("q (pl h) -> q pl h", pl=8)[:, :, 16:32],
                ALU.add)
            nc.vector.tensor_tensor(
                Q6[:].rearrange("q (pl h) -> q pl h", pl=8),
                Q5[:].rearrange("q (pl h) -> q pl h", pl=8)[:, :, 0:8],
                Q5[:].rearrange("q (pl h) -> q pl h", pl=8)[:, :, 8:16],
                ALU.add)
            nc.vector.tensor_tensor(
                Q7[:].rearrange("q (pl h) -> q pl h", pl=8),
                Q6[:].rearrange("q (pl h) -> q pl h", pl=8)[:, :, 0:4],
                Q6[:].rearrange("q (pl h) -> q pl h", pl=8)[:, :, 4:8],
                ALU.add)
            nc.vector.tensor_tensor(
                Q8[:].rearrange("q (pl h) -> q pl h", pl=8),
                Q7[:].rearrange("q (pl h) -> q pl h", pl=8)[:, :, 0:2],
                Q7[:].rearrange("q (pl h) -> q pl h", pl=8)[:, :, 2:4],
                ALU.add)
            nc.vector.tensor_tensor(
                dot8[:].rearrange("q (pl h) -> q pl h", pl=8),
                Q8[:].rearrange("q (pl h) -> q pl h", pl=8)[:, :, 0:1],
                Q8[:].rearrange("q (pl h) -> q pl h", pl=8)[:, :, 1:2],
                ALU.add)

            # softmax via exp(x) = (1+tanh(x/2))/(1-tanh(x/2))
            nc.scalar.activation(th[:], dot8[:], ACTF.Tanh, scale=0.5 * SCALE)
            nc.scalar.activation(thp1[:], th[:], ACTF.Copy, bias=1.0)
            nc.scalar.activation(om[:], th[:], ACTF.Copy, bias=1.0, scale=-1.0)
            nc.vector.reciprocal(rin[:], om[:])
            nc.vector.tensor_tensor(wexp[:], thp1[:], rin[:], ALU.mult)
            nc.vector.tensor_reduce(s8[:], wexp[:], axis=AX.X, op=ALU.add)
            nc.vector.tensor_copy(s8c[:], s8[n:128, :])
            nc.vector.tensor_tensor(ssum[:], s8[0:n, :], s8c[:], ALU.add)
            nc.vector.reciprocal(rs[:], ssum[:])

            # prefetch x for t+2
            if t + 2 < T:
                xtp = xq_pool.tile([128, 256], qdt)
                nc.sync.dma_start(xtp[:], xhd[t + 2])
                xts[t + 2] = xtp

            # weighted sum on PE: attn = sum_p w_p * A_p via stacked-diagonal
            # stationary: aps[n, h] += Dg_pl.T @ A2_pl
            for pl in range(8):
                nc.vector.tensor_scalar(Dg[:, n * pl:n * (pl + 1)], id2[:],
                                        wexp[:, pl:pl + 1], None, ALU.mult)
            aps = aps_pool.tile([n, H], f32, tag="aps")
            for pl in range(8):
                nc.tensor.matmul(aps[:], Dg[:, n * pl:n * (pl + 1)],
                                 A2[:, 512 * pl:512 * (pl + 1)],
                                 start=(pl == 0), stop=(pl == 7),
                                 skip_group_check=True)
            # evict + normalize (scale = 1/sum)
            nc.scalar.activation(attn_s[:], aps[:], ACTF.Copy, scale=rs[:])

            # attn^T chunks
            for ci in range(4):
                pt = tp_pool.tile([128, n], bf16)
                nc.tensor.transpose(pt[:], attn_s[:, 128 * ci:128 * (ci + 1)],
                                    id_sb[:])
                nc.scalar.activation(attnT[:, n * ci:n * (ci + 1)], pt[:],
                                     ACTF.Copy, scale=S_A)

            # Wattn matmuls (close each quadrant)
            for j in JORD:
                mm_w(ps, 1, j, attnT, start=False, stop=True)

            # gates (j-ordered: g, o, i computed early; f last)
            nc.scalar.activation(gg[:], ps[n:128, 512:1024], ACTF.Tanh,
                                 scale=QS)
            nc.scalar.activation(go[:], ps[0:n, 512:1024], ACTF.Sigmoid,
                                 scale=QS)
            nc.scalar.activation(gi[:], ps[0:n, 0:512], ACTF.Sigmoid, scale=QS)
            nc.vector.tensor_tensor(t2[:], gi[:], gg[:], ALU.mult)
            nc.scalar.activation(gf[:], ps[n:128, 0:512], ACTF.Sigmoid,
                                 scale=QS)
            nc.vector.tensor_tensor(t1[:], gf[:], c_st[:], ALU.mult)
            nc.vector.tensor_tensor(c_st[:], t1[:], t2[:], ALU.add)
            nc.scalar.activation(tct[:], c_st[:], ACTF.Tanh)
            nc.vector.tensor_tensor(h2[0:n, :], go[:], tct[:], ALU.mult)
            nc.vector.tensor_copy(h2[n:128, :], h2[0:n, :])

            nc.gpsimd.dma_start(hs[t], h2[0:n, :])

            for ci in range(4):
                pt = tp_pool.tile([128, n], bf16)
                nc.tensor.transpose(pt[:], h2[0:n, 128 * ci:128 * (ci + 1)],
                                    id_sb[:])
                nc.scalar.activation(hT[:, n * ci:n * (ci + 1)], pt[:],
                                     ACTF.Copy, scale=S_A)

    nc.compile()
    _cache["nc"] = nc
    return nc


LAST_RESULT = None


def kernel(x, A, Wx, Wh, Wattn, b):
    import os
    import ml_dtypes
    from concourse import bass_utils

    nc = _build_kernel()
    bft = ml_dtypes.bfloat16
    qt = ml_dtypes.float8_e4m3fn if QUANT == "fp8" else bft

    # W layout: 12 chunks of 128 rows: c 0-3 Wh, 4-7 Wattn, 8-11 Wx
    Wcat = np.concatenate([np.asarray(Wh), np.asarray(Wattn), np.asarray(Wx)],
                          axis=0)                         # (1536, 2048)
    Wc_host = np.ascontiguousarray(
        (Wcat * S_W).reshape(12, 128, H4).transpose(1, 0, 2).reshape(
            128, 12 * H4)).astype(qt)
    b_host = (np.asarray(b, dtype=np.float32) * (S_A * S_W)).reshape(
        1, H4).astype(bft)
    ident = np.eye(n, dtype=np.float32).astype(bft)
    id2_host = np.concatenate([np.eye(n), np.eye(n)], axis=0).astype(bft)
    ones_h = np.ones((1, n), dtype=bft)

    in_maps = []
    for k in range(NC):
        xc = np.asarray(x[n * k:n * (k + 1)], dtype=np.float32)   # (64, T, D)
        Ac = np.asarray(A[n * k:n * (k + 1)], dtype=np.float32)   # (64, H, 4, 4)
        xh_host = np.ascontiguousarray(
            (xc * S_A).transpose(1, 2, 0).reshape(T, 4, 128, n)
            .transpose(0, 2, 1, 3).reshape(T, 128, 4 * n)).astype(qt)
        A_flat = Ac.reshape(n, H, 16).transpose(0, 2, 1)          # (n, 16, H)
        A4 = A_flat.reshape(n, 2, 8, H)
        A2_host = np.ascontiguousarray(
            A4.transpose(1, 0, 2, 3).reshape(128, 8 * H)).astype(bft)
        in_maps.append({
            "xh": xh_host,
            "A2d": A2_host,
            "Wc": Wc_host,
            "bvec": b_host,
            "identd": ident,
            "id2d": id2_host,
            "onesd": ones_h,
        })

    trace = os.environ.get("KERNEL_TRACE") == "1"
    tmpdir = os.environ.get("KERNEL_TRACE_DIR") or None
    res = bass_utils.run_bass_kernel_spmd(
        nc, in_maps, core_ids=list(range(NC)), trace=trace, tmpdir=tmpdir
    )
    global LAST_RESULT
    LAST_RESULT = res

    out = np.empty((N, T, H), dtype=np.float32)
    for k in range(NC):
        hs_k = np.asarray(res.results[k]["hs"])           # (T, n, H) bf16
        out[n * k:n * (k + 1)] = hs_k.transpose(1, 0, 2).astype(np.float32)
    return out


# revision 18
# speedup vs baseline: 1.1501x; 1.0266x over previous
import math
from contextlib import ExitStack

import numpy as np

N, T, D, H = 512, 128, 512, 512
NC = 8
n = N // NC          # 64 samples per core
H4 = 4 * H           # 2048
SCALE = 1.0 / math.sqrt(H)

# QUANT: "bf16" or "fp8" (fp8 = DoubleRow for Wh/Wattn/Wx matmuls)
QUANT = "fp8"
S_A = 8.0 if QUANT == "fp8" else 1.0    # stationary (activations) scale
S_W = 16.0 if QUANT == "fp8" else 1.0   # weight scale
QS = 1.0 / (S_A * S_W)                  # gate descale

_cache = {}


def _build_kernel(has_bias):
    key = ("nc", has_bias)
    if key in _cache:
        return _cache[key]

    import concourse.bass as bass
    import concourse.tile as tile
    from concourse import bacc, mybir

    f32 = mybir.dt.float32
    bf16 = mybir.dt.bfloat16
    fp8 = mybir.dt.float8e4
    FP8 = QUANT == "fp8"
    qdt = fp8 if FP8 else bf16
    ALU = mybir.AluOpType
    ACTF = mybir.ActivationFunctionType
    AX = mybir.AxisListType
    DR = mybir.MatmulPerfMode.DoubleRow if FP8 else None
    # padded stationary tiles in fp8: [128, (4 kt, 128)] with data at
    # [kt, 64:128]; zeros at [kt, 0:64] (DoubleRow cannot use tile_position,
    # so rows 64:128 of psum are written via a zero-padded full-width lhsT)
    AW = 128 if FP8 else 64   # per-chunk stationary width

    nc = bacc.Bacc(
        "TRN2",
        target_bir_lowering=False,
        debug=False,
        enable_asserts=False,
        num_devices=NC,
    )

    xhd = nc.dram_tensor("xh", (T, 128, 256), qdt, kind="ExternalInput").ap()
    A2d = nc.dram_tensor("A2d", (128, 8 * H), bf16, kind="ExternalInput").ap()
    Wcd = nc.dram_tensor("Wc", (128, 12 * H4), qdt, kind="ExternalInput").ap()
    bd = nc.dram_tensor("bvec", (1, H4), bf16, kind="ExternalInput").ap()
    idd = nc.dram_tensor("identd", (n, n), bf16, kind="ExternalInput").ap()
    id2d = nc.dram_tensor("id2d", (128, n), bf16, kind="ExternalInput").ap()
    onesd = nc.dram_tensor("onesd", (1, n), bf16, kind="ExternalInput").ap()
    hs = nc.dram_tensor("hs", (T, n, H), bf16, kind="ExternalOutput").ap()

    with tile.TileContext(nc) as tc, ExitStack() as ctx:
        const_pool = ctx.enter_context(tc.tile_pool(name="const", bufs=1))
        gps_pool = ctx.enter_context(tc.tile_pool(name="gps", bufs=2, space="PSUM"))
        tp_pool = ctx.enter_context(tc.tile_pool(name="tp", bufs=3, space="PSUM"))
        aps_pool = ctx.enter_context(tc.tile_pool(name="aps", bufs=1, space="PSUM"))

        # ---- persistent tiles --------------------------------------------
        W_sb = const_pool.tile([128, 12 * H4], qdt)
        b_sb = const_pool.tile([1, H4], bf16)
        id_sb = const_pool.tile([n, n], bf16)    # x S_A (transpose identity)
        id2 = const_pool.tile([128, n], bf16)    # [I; I] plain
        ones_row = const_pool.tile([1, n], bf16)
        A2 = const_pool.tile([128, 8 * H], bf16)     # q=(ph,s); free=(pl, h)
        nc.sync.dma_start(W_sb[:], Wcd[:])
        nc.sync.dma_start(b_sb[:], bd[:])
        nc.sync.dma_start(id_sb[:], idd[:])
        nc.sync.dma_start(id2[:], id2d[:])
        nc.sync.dma_start(ones_row[:], onesd[:])
        nc.sync.dma_start(A2[:], A2d[:])

        h2 = const_pool.tile([128, H], bf16)         # h duplicated on both halves
        c_st = const_pool.tile([n, H], bf16)
        hT = const_pool.tile([128, 4 * AW], qdt)     # h^T chunks (x S_A)
        attnT = const_pool.tile([128, 4 * AW], qdt)  # attn^T chunks (x S_A)
        xqb = [const_pool.tile([128, 4 * AW], qdt, name=f"xqb{i}")
               for i in range(3)]
        if FP8:
            # zero the pad columns once; DMA/copies only touch data slots
            for tl in (hT, attnT, *xqb):
                nc.vector.memset(
                    tl[:].rearrange("q (t w) -> q t w", t=4)[:, :, 0:64], 0)

        # per-step scratch
        P = const_pool.tile([128, 8 * H], bf16)      # A2 * h products
        Q1 = const_pool.tile([128, 2048], bf16)      # h-fold tree levels
        Q2 = const_pool.tile([128, 1024], bf16)
        Q3 = const_pool.tile([128, 512], bf16)
        Q4 = const_pool.tile([128, 256], bf16)
        Q5 = const_pool.tile([128, 128], bf16)
        dot8 = const_pool.tile([128, 8], f32)
        th = const_pool.tile([128, 8], f32)
        thp1 = const_pool.tile([128, 8], bf16)
        om = const_pool.tile([128, 8], f32)
        rin = const_pool.tile([128, 8], f32)
        wexp = const_pool.tile([128, 8], f32)
        Dg = const_pool.tile([128, 8 * n], bf16)     # 8 diag(w_p-pair) tiles
        s8 = const_pool.tile([128, 1], f32)
        s8c = const_pool.tile([n, 1], f32)
        ssum = const_pool.tile([n, 1], f32)
        rs = const_pool.tile([n, 1], f32)
        attn_s = const_pool.tile([n, H], bf16)
        gi = const_pool.tile([n, H], bf16)
        gf = const_pool.tile([n, H], bf16)
        go = const_pool.tile([n, H], bf16)
        gg = const_pool.tile([n, H], bf16)
        t1 = const_pool.tile([n, H], bf16)
        t2 = const_pool.tile([n, H], bf16)
        tct = const_pool.tile([n, H], bf16)
        i0 = const_pool.tile([128, 4 * H], f32)      # h0 init scratch
        i1 = const_pool.tile([128, 2 * H], f32)
        h0h = const_pool.tile([n, H], f32)

        # gate psum: ps_if [128,512]: rows 0:64 = i, 64:128 = f (cols 0:512
        # of hbar are i, 512:1024 f); ps_og: rows 0:64 = o, 64:128 = g.
        def bank(pst, j):
            # returns (out_ap, zero_padded?, tile_position or None)
            ps_if, ps_og = pst[0], pst[1]
            ps = ps_if if j in (0, 1) else ps_og
            if j in (0, 2):
                return ps[0:n, :], False, (0, 0)
            if FP8:
                return ps[:, :], True, None
            return ps[n:128, :], False, (0, n)

        def mark_start(pst, j):
            keyb = "if" if j in (0, 1) else "og"
            if keyb in pst[2]:
                return False
            pst[2].add(keyb)
            return True

        def mm_w(pst, m, j, lhsT8, stop):
            # weight matmul against matrix m (0=Wh, 1=Wattn, 2=Wx)
            out, zp, tpos = bank(pst, j)
            if FP8:
                lv = lhsT8[:].rearrange("q (t w) -> q t w", t=4)
                for cp in range(2):
                    rhs = W_sb[:, H4 * 4 * m + H4 * 2 * cp:
                               H4 * 4 * m + H4 * 2 * (cp + 1)].rearrange(
                        "q (t c) -> q t c", t=2)[:, :, 512 * j:512 * (j + 1)]
                    lhsT = lv[:, 2 * cp:2 * cp + 2, :] if zp else \
                        lv[:, 2 * cp:2 * cp + 2, 64:128]
                    nc.tensor.matmul(out, lhsT, rhs,
                                     start=(cp == 0 and mark_start(pst, j)),
                                     stop=(stop and cp == 1),
                                     perf_mode=DR,
                                     skip_group_check=True)
            else:
                for c in range(4):
                    rhs = W_sb[:, H4 * (4 * m + c) + 512 * j:
                               H4 * (4 * m + c) + 512 * (j + 1)]
                    lhsT = lhsT8[:, n * c:n * (c + 1)]
                    nc.tensor.matmul(out, lhsT, rhs,
                                     start=(c == 0 and mark_start(pst, j)),
                                     stop=(stop and c == 3),
                                     tile_position=tpos,
                                     skip_group_check=True)

        def mm_bias(pst, j):
            ps_if, ps_og = pst[0], pst[1]
            ps = ps_if if j in (0, 1) else ps_og
            out = ps[0:n, :] if j in (0, 2) else ps[n:128, :]
            tpos = (0, 0) if j in (0, 2) else (0, n)
            nc.tensor.matmul(out, ones_row[:], b_sb[:, 512 * j:512 * (j + 1)],
                             start=mark_start(pst, j), stop=False,
                             tile_position=tpos, skip_group_check=True)

        JORD = (3, 2, 0, 1)   # g, o, i, f
        pss = {}

        def emit_biasx(t):
            ps_if = gps_pool.tile([128, 512], f32, tag="psif")
            ps_og = gps_pool.tile([128, 512], f32, tag="psog")
            pst = (ps_if, ps_og, set())
            pss[t] = pst
            for j in JORD:
                if has_bias:
                    mm_bias(pst, j)
                mm_w(pst, 2, j, xqb[t % 3], stop=False)
            return pst

        def emit_wh(t):
            for j in JORD:
                mm_w(pss[t], 0, j, hT, stop=False)

        def emit_hT():
            for ci in range(4):
                pt = tp_pool.tile([128, n], bf16)
                nc.tensor.transpose(pt[:], h2[0:n, 128 * ci:128 * (ci + 1)],
                                    id_sb[:])
                nc.scalar.copy(hT[:, AW * ci + AW - 64:AW * (ci + 1)], pt[:])

        def load_x(t):
            if FP8:
                dst = xqb[t % 3][:].rearrange("q (t w) -> q t w",
                                              t=4)[:, :, 64:128]
                nc.sync.dma_start(
                    dst, xhd[t].rearrange("q (t w) -> q t w", t=4))
            else:
                nc.sync.dma_start(xqb[t % 3][:], xhd[t])

        # ---- h0 = mean over p of A_flat; c0 = h0 -------------------------
        nc.vector.tensor_tensor(i0[:], A2[:, 0:2048], A2[:, 2048:4096], ALU.add)
        nc.vector.tensor_tensor(i1[:], i0[:, 0:1024], i0[:, 1024:2048], ALU.add)
        nc.vector.tensor_tensor(i0[:, 0:512], i1[:, 0:512], i1[:, 512:1024],
                                ALU.add)
        nc.vector.tensor_copy(i1[0:n, 0:512], i0[n:128, 0:512])
        nc.vector.tensor_tensor(h0h[:], i0[0:n, 0:512], i1[0:n, 0:512],
                                ALU.add)
        nc.scalar.activation(c_st[:], h0h[:], ACTF.Copy, scale=1.0 / 16.0)
        nc.scalar.activation(h2[0:n, :], h0h[:], ACTF.Copy, scale=1.0 / 16.0)
        nc.vector.tensor_copy(h2[n:128, :], h2[0:n, :])

        load_x(0)
        load_x(1)
        emit_hT()
        emit_biasx(0)
        emit_wh(0)

        Pv = P[:].rearrange("q (pl h) -> q pl h", pl=8)

        # ---- recurrence ---------------------------------------------------
        for t in range(T):
            pst = pss.pop(t)
            ps_if, ps_og = pst[0], pst[1]

            # attention dot: P = A2 * h2 (broadcast over pl), fold h by tree
            nc.vector.tensor_tensor(
                Pv, A2[:].rearrange("q (pl h) -> q pl h", pl=8),
                h2[:].rearrange("q (r h) -> q r h", r=1).broadcast_to(
                    [128, 8, H]),
                ALU.mult)
            nc.vector.tensor_tensor(
                Q1[:].rearrange("q (pl h) -> q pl h", pl=8),
                Pv[:, :, 0:256], Pv[:, :, 256:512], ALU.add)
            nc.vector.tensor_tensor(
                Q2[:].rearrange("q (pl h) -> q pl h", pl=8),
                Q1[:].rearrange("q (pl h) -> q pl h", pl=8)[:, :, 0:128],
                Q1[:].rearrange("q (pl h) -> q pl h", pl=8)[:, :, 128:256],
                ALU.add)
            nc.vector.tensor_tensor(
                Q3[:].rearrange("q (pl h) -> q pl h", pl=8),
                Q2[:].rearrange("q (pl h) -> q pl h", pl=8)[:, :, 0:64],
                Q2[:].rearrange("q (pl h) -> q pl h", pl=8)[:, :, 64:128],
                ALU.add)
            nc.vector.tensor_tensor(
                Q4[:].rearrange("q (pl h) -> q pl h", pl=8),
                Q3[:].rearrange("q (pl h) -> q pl h", pl=8)[:, :, 0:32],
                Q3[:].rearrange("q (pl h) -> q pl h", pl=8)[:, :, 32:64],
                ALU.add)
            nc.vector.tensor_tensor(
                Q5[:].rearrange("q (pl h) -> q pl h", pl=8),
                Q4[:].rearrange("q (pl h) -> q pl h", pl=8)[:, :, 0:16],
                Q4[:].rearrange("q (pl h) -> q pl h", pl=8)[:, :, 16:32],
                ALU.add)
            nc.vector.tensor_reduce(
                dot8[:], Q5[:].rearrange("q (pl h) -> q pl h", pl=8),
                axis=AX.X, op=ALU.add)

            # softmax via exp(x) = (1+tanh(x/2))/(1-tanh(x/2))
            nc.scalar.activation(th[:], dot8[:], ACTF.Tanh, scale=0.5 * SCALE)
            nc.vector.tensor_scalar(thp1[:], th[:], 1.0, None, ALU.add)
            nc.vector.tensor_scalar(om[:], th[:], -1.0, 1.0, ALU.mult, ALU.add)
            nc.vector.reciprocal(rin[:], om[:])
            nc.vector.tensor_tensor(wexp[:], thp1[:], rin[:], ALU.mult)
            nc.vector.tensor_reduce(s8[:], wexp[:], axis=AX.X, op=ALU.add)
            nc.vector.tensor_copy(s8c[:], s8[n:128, :])
            nc.vector.tensor_tensor(ssum[:], s8[0:n, :], s8c[:], ALU.add)
            nc.vector.reciprocal(rs[:], ssum[:])

            # weighted sum on PE: attn = sum_p w_p * A_p via stacked-diagonal
            nc.vector.tensor_tensor(
                Dg[:].rearrange("q (pl w) -> q pl w", pl=8),
                id2[:].rearrange("q (r w) -> q r w", r=1).broadcast_to(
                    [128, 8, n]),
                wexp[:].rearrange("q (pl r) -> q pl r", r=1).broadcast_to(
                    [128, 8, n]),
                ALU.mult)
            aps = aps_pool.tile([n, H], f32, tag="aps")
            for pl in range(8):
                nc.tensor.matmul(aps[:], Dg[:, n * pl:n * (pl + 1)],
                                 A2[:, 512 * pl:512 * (pl + 1)],
                                 start=(pl == 0), stop=(pl == 7),
                                 skip_group_check=True)
            # evict + normalize (scale = 1/sum) on DVE
            nc.vector.tensor_scalar(attn_s[:], aps[:], rs[:], None, ALU.mult)

            # attn^T chunks (identity is pre-scaled by S_A)
            for ci in range(4):
                pt = tp_pool.tile([128, n], bf16)
                nc.tensor.transpose(pt[:], attn_s[:, 128 * ci:128 * (ci + 1)],
                                    id_sb[:])
                nc.scalar.copy(attnT[:, AW * ci + AW - 64:AW * (ci + 1)],
                               pt[:])

            # Wattn matmuls (close each bank)
            for j in JORD:
                mm_w(pst, 1, j, attnT, stop=(j in (2, 1)))

            # gates (j-ordered: g, o, i computed early; f last)
            gsc = {} if QS == 1.0 else {"scale": QS}
            nc.scalar.activation(gg[:], ps_og[n:128, :], ACTF.Tanh, **gsc)
            nc.scalar.activation(go[:], ps_og[0:n, :], ACTF.Sigmoid, **gsc)
            nc.scalar.activation(gi[:], ps_if[0:n, :], ACTF.Sigmoid, **gsc)
            nc.gpsimd.tensor_tensor(t2[:], gi[:], gg[:], ALU.mult)
            nc.scalar.activation(gf[:], ps_if[n:128, :], ACTF.Sigmoid, **gsc)
            nc.vector.tensor_tensor(t1[:], gf[:], c_st[:], ALU.mult)
            nc.vector.tensor_tensor(c_st[:], t1[:], t2[:], ALU.add)
            nc.scalar.activation(tct[:], c_st[:], ACTF.Tanh)
            nc.vector.tensor_tensor(h2[0:n, :], go[:], tct[:], ALU.mult)
            nc.vector.tensor_copy(h2[n:128, :], h2[0:n, :])

            nc.gpsimd.dma_start(hs[t], h2[0:n, :])

            # preload next step's PE work while gates tail runs (keeps PE warm)
            if t + 1 < T:
                emit_biasx(t + 1)
                emit_hT()
                emit_wh(t + 1)
            if t + 2 < T:
                load_x(t + 2)

    nc.compile()
    _cache[key] = nc
    return nc


LAST_RESULT = None


def kernel(x, A, Wx, Wh, Wattn, b):
    import os
    import ml_dtypes
    from concourse import bass_utils

    has_bias = bool(np.any(np.asarray(b)))
    nc = _build_kernel(has_bias)
    bft = ml_dtypes.bfloat16
    qt = ml_dtypes.float8_e4m3fn if QUANT == "fp8" else bft

    # W layout: 12 chunks of 128 rows: c 0-3 Wh, 4-7 Wattn, 8-11 Wx
    Wcat = np.concatenate([np.asarray(Wh), np.asarray(Wattn), np.asarray(Wx)],
                          axis=0)                         # (1536, 2048)
    Wc_host = np.ascontiguousarray(
        (Wcat * S_W).reshape(12, 128, H4).transpose(1, 0, 2).reshape(
            128, 12 * H4)).astype(qt)
    b_host = (np.asarray(b, dtype=np.float32) * (S_A * S_W)).reshape(
        1, H4).astype(bft)
    ident = (np.eye(n, dtype=np.float32) * S_A).astype(bft)
    id2_host = np.concatenate([np.eye(n), np.eye(n)], axis=0).astype(bft)
    ones_h = np.ones((1, n), dtype=bft)

    in_maps = []
    for k in range(NC):
        xc = np.asarray(x[n * k:n * (k + 1)], dtype=np.float32)   # (64, T, D)
        Ac = np.asarray(A[n * k:n * (k + 1)], dtype=np.float32)   # (64, H, 4, 4)
        xh_host = np.ascontiguousarray(
            (xc * S_A).transpose(1, 2, 0).reshape(T, 4, 128, n)
            .transpose(0, 2, 1, 3).reshape(T, 128, 4 * n)).astype(qt)
        A_flat = Ac.reshape(n, H, 16).transpose(0, 2, 1)          # (n, 16, H)
        A4 = A_flat.reshape(n, 2, 8, H)
        A2_host = np.ascontiguousarray(
            A4.transpose(1, 0, 2, 3).reshape(128, 8 * H)).astype(bft)
        in_maps.append({
            "xh": xh_host,
            "A2d": A2_host,
            "Wc": Wc_host,
            "bvec": b_host,
            "identd": ident,
            "id2d": id2_host,
            "onesd": ones_h,
        })

    trace = os.environ.get("KERNEL_TRACE") == "1"
    tmpdir = os.environ.get("KERNEL_TRACE_DIR") or None
    res = bass_utils.run_bass_kernel_spmd(
        nc, in_maps, core_ids=list(range(NC)), trace=trace, tmpdir=tmpdir
    )
    global LAST_RESULT
    LAST_RESULT = res

    out = np.empty((N, T, H), dtype=np.float32)
    for k in range(NC):
        hs_k = np.asarray(res.results[k]["hs"])           # (T, n, H) bf16
        out[n * k:n * (k + 1)] = hs_k.transpose(1, 0, 2).astype(np.float32)
    return out
